# revision 1
# baseline (speedup 1.0000x reference)
"""BiRNN language model on 8 Trainium2 NeuronCores.

Model (see reference): emb lookup -> two tiny 16-wide RNNs (L->R and R->L,
collecting pre-update states) -> logits = [hLR|hRL] @ W_ho.T + b_ho over a
50257 vocab -> log_softmax.  Output [64, 32, 50257] f32 (~412 MB) dominates:
memory-bound regime.

Sharding: data-parallel over batch (B=32 -> 4 columns/core).  Each core:
  1. gathers its 256 embedding rows (indirect DMA), PE-transposes them,
  2. precomputes xproj = W_x @ x + b for every step in one matmul, then runs
     both recurrences with one small K=16 matmul + tanh per step (psum
     prefilled with xproj via DVE so bias/input-proj cost nothing per step),
  3. streams W_aug = [W_ho.T; b_ho] (33 x Vpad, f32r) from HBM in 8-chunk
     (16KB/partition) DMAs; logits = fp32r matmuls against
     haug = [hLR; hRL; 1] (bias folded in via the ones row),
  4. log-softmax without max-subtraction (|logits| <= ~8.5 so exp is safe):
     pass 1 computes exp (bf16) on ACT + per-chunk row-sums on DVE;
     pass 2 recomputes logits and applies the per-row -ln(sum) while copying
     psum->SBUF (alternating ACT/DVE), staging 8 chunks per 16KB-descriptor
     DMA to HBM.
No collectives needed; the host concatenates the 8 batch slices.
"""

import sys

sys.path.insert(0, "/opt/trn_rl_repo")

from contextlib import ExitStack

import numpy as np

import concourse.bass as bass
import concourse.bacc as bacc
import concourse.tile as tile
from concourse import mybir
from concourse.bass_utils import run_bass_kernel_spmd
from concourse.masks import make_identity

S, B, V, HID, EMB = 64, 32, 50257, 16, 32
NCORES = 8
BL = B // NCORES          # batch columns per core
R = S * BL                # logit rows per core
XA = EMB + 1              # 33: [x; 1] contraction for the xproj precompute
KA = 2 * HID + 1          # 33: [hLR; hRL; 1] contraction for logits
CHUNK = 512               # vocab columns per PSUM tile
NCHUNKS = (V + CHUNK - 1) // CHUNK
# fp32r matmuls require even free sizes; pad the vocab to a whole number of
# chunks.  Pad columns get bias -1e4 so exp(logit) == 0 and the row sums are
# unaffected; they are never written to the output.
VPAD = NCHUNKS * CHUNK
GRP = 8                   # chunks per W-load / output-store DMA group

f32 = mybir.dt.float32
f32r = mybir.dt.float32r
bf16 = mybir.dt.bfloat16
i32 = mybir.dt.int32
AF = mybir.ActivationFunctionType


def build_nc():
    nc = bacc.Bacc()

    ind = nc.declare_dram_parameter("ind", [R, 1], i32, isOutput=False)
    emb_tab = nc.declare_dram_parameter("emb_tab", [V, EMB], f32, isOutput=False)
    # [W_x.T; b] per direction for the xproj precompute
    wlrx = nc.declare_dram_parameter("wlrx", [XA, HID], f32, isOutput=False)
    wrlx = nc.declare_dram_parameter("wrlx", [XA, HID], f32, isOutput=False)
    # W_h.T per direction for the per-step recurrence matmul
    wlrh = nc.declare_dram_parameter("wlrh", [HID, HID], f32, isOutput=False)
    wrlh = nc.declare_dram_parameter("wrlh", [HID, HID], f32, isOutput=False)
    h0c = nc.declare_dram_parameter("h0c", [HID, BL], f32, isOutput=False)
    waug = nc.declare_dram_parameter("waug", [KA, VPAD], f32r, isOutput=False)
    waug_bf = nc.declare_dram_parameter("waug_bf", [KA, VPAD], bf16, isOutput=False)
    out = nc.declare_dram_parameter("out", [R, V], f32, isOutput=True)

    groups = [(g0, min(GRP, NCHUNKS - g0)) for g0 in range(0, NCHUNKS, GRP)]

    with ExitStack() as ctx:
        tc = ctx.enter_context(tile.TileContext(nc))
        consts = ctx.enter_context(tc.tile_pool(name="consts", bufs=1))
        wpool = ctx.enter_context(tc.tile_pool(name="wpool", bufs=2))
        epool = ctx.enter_context(tc.tile_pool(name="epool", bufs=4))
        opool = ctx.enter_context(tc.tile_pool(name="opool", bufs=3))
        pbig = ctx.enter_context(tc.tile_pool(name="pbig", bufs=4, space="PSUM"))
        psmall = ctx.enter_context(tc.tile_pool(name="psmall", bufs=2, space="PSUM"))

        # ---- gather embeddings for this core's 256 (step, batch) rows ----
        idx0 = consts.tile([128, 1], i32, tag="idx0")
        idx1 = consts.tile([128, 1], i32, tag="idx1")
        nc.sync.dma_start(out=idx0[:, :], in_=ind[0:128, :])
        nc.sync.dma_start(out=idx1[:, :], in_=ind[128:256, :])
        er0 = consts.tile([128, EMB], f32, tag="er0")
        er1 = consts.tile([128, EMB], f32, tag="er1")
        nc.gpsimd.indirect_dma_start(
            out=er0[:, :], out_offset=None, in_=emb_tab[:, :],
            in_offset=bass.IndirectOffsetOnAxis(ap=idx0[:, :1], axis=0))
        nc.gpsimd.indirect_dma_start(
            out=er1[:, :], out_offset=None, in_=emb_tab[:, :],
            in_offset=bass.IndirectOffsetOnAxis(ap=idx1[:, :1], axis=0))

        # ---- transpose to x-major layout: xa = [X (32 rows); ones] ----
        ident = consts.tile([128, 128], f32, tag="ident")
        make_identity(nc, ident)
        xa = consts.tile([XA, R], f32, tag="xa")
        nc.vector.memset(xa[EMB:XA, :], 1.0)
        for half, er in ((0, er0), (1, er1)):
            pt = pbig.tile([EMB, 128], f32, tag="pb")
            nc.tensor.transpose(pt[:, :], er[:, :], ident[:, :])
            nc.vector.tensor_copy(out=xa[0:EMB, half * 128:(half + 1) * 128],
                                  in_=pt[:, :])

        # ---- RNN parameters ----
        wlrx_s = consts.tile([XA, HID], f32, tag="wlrx")
        wrlx_s = consts.tile([XA, HID], f32, tag="wrlx")
        wlrh_s = consts.tile([HID, HID], f32, tag="wlrh")
        wrlh_s = consts.tile([HID, HID], f32, tag="wrlh")
        for dst, src in ((wlrx_s, wlrx), (wrlx_s, wrlx),
                         (wlrh_s, wlrh), (wrlh_s, wrlh)):
            nc.sync.dma_start(out=dst[:, :], in_=src[:, :])

        # ---- xproj precompute: xp = W_x @ x + b for all steps ----
        xp_lr = consts.tile([HID, R], f32, tag="xp_lr")
        xp_rl = consts.tile([HID, R], f32, tag="xp_rl")
        for xp, w in ((xp_lr, wlrx_s), (xp_rl, wrlx_s)):
            pp = pbig.tile([HID, R], f32, tag="pb")
            nc.tensor.matmul(pp[:, :], lhsT=w[:, :], rhs=xa[:, :],
                             start=True, stop=True)
            nc.vector.tensor_copy(out=xp[:, :], in_=pp[:, :])

        # ---- the two recurrences ----
        # h_lr block i (cols 4i:4i+4) = hLR_pre[i]; block 0 = h0.
        # h_rl block b+1 = hRL_pre[b]; block 64 = h0 (pre-state of word 63).
        h_lr = consts.tile([HID, BL * (S + 1)], f32, tag="h_lr")
        h_rl = consts.tile([HID, BL * (S + 1)], f32, tag="h_rl")
        nc.sync.dma_start(out=h_lr[:, 0:BL], in_=h0c[:, :])
        nc.sync.dma_start(out=h_rl[:, S * BL:(S + 1) * BL], in_=h0c[:, :])
        # Interleave the two chains with separate psum tags so their pool
        # slots rotate independently (a shared tag serializes the chains).
        for i in range(S):
            w = S - 1 - i
            ps = psmall.tile([HID, BL], f32, tag="sp_lr", name=f"pl{i}")
            nc.vector.tensor_copy(out=ps[:, :], in_=xp_lr[:, i * BL:(i + 1) * BL])
            nc.tensor.matmul(ps[:, :], lhsT=wlrh_s[:, :],
                             rhs=h_lr[:, i * BL:(i + 1) * BL],
                             start=False, stop=True, skip_group_check=True)
            nc.scalar.activation(out=h_lr[:, (i + 1) * BL:(i + 2) * BL],
                                 in_=ps[:, :], func=AF.Tanh)
            ps2 = psmall.tile([HID, BL], f32, tag="sp_rl", name=f"pr{i}")
            nc.vector.tensor_copy(out=ps2[:, :], in_=xp_rl[:, w * BL:(w + 1) * BL])
            nc.tensor.matmul(ps2[:, :], lhsT=wrlh_s[:, :],
                             rhs=h_rl[:, (w + 1) * BL:(w + 2) * BL],
                             start=False, stop=True, skip_group_check=True)
            nc.scalar.activation(out=h_rl[:, w * BL:(w + 1) * BL],
                                 in_=ps2[:, :], func=AF.Tanh)

        # ---- h_aug = [hLR; hRL; 1] as [33, 256] (matmul lhsT layout) ----
        # Rows 16:32 aren't a legal compute-engine write target (partition
        # start must be 0/32/64/96) but DMA can write there.
        haug = consts.tile([KA, R], f32r, tag="haug")
        ones = consts.tile([1, R], f32, tag="ones")
        nc.vector.memset(ones[:, :], 1.0)
        nc.vector.tensor_copy(out=haug[0:HID, :], in_=h_lr[:, 0:R])
        nc.sync.dma_start(out=haug[HID:2 * HID, :],
                          in_=h_rl[:, BL:R + BL].bitcast(f32r))
        nc.vector.tensor_copy(out=haug[2 * HID:KA, :], in_=ones[:, :])
        # bf16 shadow of haug for pass 1: the exp-sum averages out bf16
        # rounding across 50k terms, so ln(sum) is unaffected.
        haug_bf = consts.tile([KA, R], bf16, tag="haug_bf")
        nc.vector.tensor_copy(out=haug_bf[:, :], in_=haug[:, :].bitcast(f32))

        # ---- pass 1: per-row sum(exp(logits)) ----
        sums = [consts.tile([128, NCHUNKS], f32, tag=f"sums{rc}", name=f"sums{rc}")
                for rc in range(2)]
        for g0, ng in groups:
            wtb = wpool.tile([KA, GRP * CHUNK], bf16, tag="wtb")
            nc.sync.dma_start(out=wtb[:, :ng * CHUNK],
                              in_=waug_bf[:, g0 * CHUNK:(g0 + ng) * CHUNK])
            for k in range(ng):
                j = g0 + k
                for rc in range(2):
                    ps = pbig.tile([128, CHUNK], f32, tag="pb")
                    nc.tensor.matmul(ps[:, :],
                                     lhsT=haug_bf[:, rc * 128:(rc + 1) * 128],
                                     rhs=wtb[:, k * CHUNK:(k + 1) * CHUNK],
                                     start=True, stop=True)
                    ex = epool.tile([128, CHUNK], bf16, tag="ex")
                    nc.scalar.activation(out=ex[:, :], in_=ps[:, :], func=AF.Exp)
                    nc.vector.reduce_sum(out=sums[rc][:, j:j + 1], in_=ex[:, :],
                                         axis=mybir.AxisListType.X)

        # ---- -ln(sum) per row ----
        negl = []
        for rc in range(2):
            tot = consts.tile([128, 1], f32, tag=f"tot{rc}", name=f"tot{rc}")
            nc.vector.reduce_sum(out=tot[:, :], in_=sums[rc][:, 0:NCHUNKS],
                                 axis=mybir.AxisListType.X)
            ln = consts.tile([128, 1], f32, tag=f"ln{rc}", name=f"ln{rc}")
            nc.scalar.activation(out=ln[:, :], in_=tot[:, :], func=AF.Ln)
            ng_t = consts.tile([128, 1], f32, tag=f"ng{rc}", name=f"ng{rc}")
            nc.vector.tensor_scalar_mul(out=ng_t[:, :], in0=ln[:, :], scalar1=-1.0)
            negl.append(ng_t)

        # ---- pass 2: recompute logits, subtract ln(sum) into staged tiles
        # (alternating ACT/DVE), stream 8-chunk groups to HBM ----
        for g0, ng in groups:
            gw = ng * CHUNK
            c0 = g0 * CHUNK
            wt = wpool.tile([KA, GRP * CHUNK], bf16, tag="wt")
            nc.sync.dma_start(out=wt[:, :gw], in_=waug_bf[:, c0:c0 + gw])
            obig = [opool.tile([128, GRP * CHUNK], f32, tag=f"ob{rc}",
                               name=f"ob{rc}_{g0}") for rc in range(2)]
            for k in range(ng):
                j = g0 + k
                for rc in range(2):
                    ps = pbig.tile([128, CHUNK], f32, tag="pb")
                    nc.tensor.matmul(ps[:, :],
                                     lhsT=haug_bf[:, rc * 128:(rc + 1) * 128],
                                     rhs=wt[:, k * CHUNK:(k + 1) * CHUNK],
                                     start=True, stop=True)
                    dst = obig[rc][:, k * CHUNK:(k + 1) * CHUNK]
                    if (2 * j + rc) % 2 == 0:
                        nc.scalar.activation(out=dst, in_=ps[:, :],
                                             func=AF.Identity,
                                             bias=negl[rc][:, 0:1])
                    else:
                        nc.vector.tensor_scalar_add(out=dst, in0=ps[:, :],
                                                    scalar1=negl[rc][:, 0:1])
            cw = min(gw, V - c0)
            for rc in range(2):
                nc.sync.dma_start(out=out[rc * 128:(rc + 1) * 128, c0:c0 + cw],
                                  in_=obig[rc][:, :cw])
    nc.finalize()
    return nc


_NC = None


def get_nc():
    global _NC
    if _NC is None:
        _NC = build_nc()
    return _NC


def _make_waug(Who, bho):
    # Matches the haug partition layout: [W_hLR; W_hRL; b_ho].
    # Pad columns carry bias -1e4 so exp(logit) underflows to exactly 0.
    waug = np.zeros((KA, VPAD), dtype=np.float32)
    waug[0:2 * HID, :V] = Who.T
    waug[2 * HID, :V] = bho
    waug[2 * HID, V:] = -1e4
    return waug


def make_in_maps(**inputs):
    ib = np.asarray(inputs["input_batch"]).astype(np.int32)          # [S, B]
    emb = np.ascontiguousarray(np.asarray(inputs["embedding"], dtype=np.float32))
    Wlr = np.asarray(inputs["W_lr"], dtype=np.float32)               # [16, 48]
    Wrl = np.asarray(inputs["W_rl"], dtype=np.float32)
    blr = np.asarray(inputs["b_lr"], dtype=np.float32).reshape(1, HID)
    brl = np.asarray(inputs["b_rl"], dtype=np.float32).reshape(1, HID)
    Who = np.asarray(inputs["W_ho"], dtype=np.float32)               # [V, 32]
    bho = np.asarray(inputs["b_ho"], dtype=np.float32)               # [V]
    h0 = np.asarray(inputs["h0"], dtype=np.float32)                  # [1, 16]

    waug = _make_waug(Who, bho)
    shared = dict(
        emb_tab=emb,
        waug_bf=waug.astype(mybir.dt.np(bf16)),
        wlrx=np.ascontiguousarray(np.concatenate([Wlr[:, :EMB].T, blr], axis=0)),
        wrlx=np.ascontiguousarray(np.concatenate([Wrl[:, :EMB].T, brl], axis=0)),
        wlrh=np.ascontiguousarray(Wlr[:, EMB:].T),
        wrlh=np.ascontiguousarray(Wrl[:, EMB:].T),
        h0c=np.ascontiguousarray(np.broadcast_to(h0.T, (HID, BL))),
        waug=waug,
    )
    in_maps = []
    for c in range(NCORES):
        ind = np.ascontiguousarray(
            ib[:, c * BL:(c + 1) * BL].reshape(R, 1))
        in_maps.append({**shared, "ind": ind})
    return in_maps


def assemble(results):
    outs = [results[c]["out"].reshape(S, BL, V) for c in range(NCORES)]
    return np.concatenate(outs, axis=1)


def kernel(**inputs):
    in_maps = make_in_maps(**inputs)
    res = run_bass_kernel_spmd(get_nc(), in_maps, list(range(NCORES)))
    return assemble(res.results)


if __name__ == "__main__":
    rng = np.random.default_rng(0)
    stdv = 1.0 / np.sqrt(HID)
    u = lambda *shp: rng.uniform(-stdv, stdv, shp).astype(np.float32)
    demo = dict(
        input_batch=rng.integers(0, V, (S, B)).astype(np.int32),
        embedding=u(V, EMB), W_lr=u(HID, EMB + HID), b_lr=u(HID),
        W_rl=u(HID, EMB + HID), b_rl=u(HID), W_ho=u(V, 2 * HID), b_ho=u(V),
        h0=u(1, HID),
    )
    out_arr = kernel(**demo)
    print(out_arr.shape, out_arr.dtype, float(out_arr[0, 0, :3].sum()))



# revision 5
# speedup vs baseline: 1.2933x; 1.2933x over previous
"""BiRNN language model on 8 Trainium2 NeuronCores.

Model (see reference): emb lookup -> two tiny 16-wide RNNs (L->R and R->L,
collecting pre-update states) -> logits = [hLR|hRL] @ W_ho.T + b_ho over a
50257 vocab -> log_softmax.  Output [64, 32, 50257] (~412 MB) dominates:
memory-bound regime.

Sharding: data-parallel over batch (B=32 -> 4 columns/core).  Each core:
  1. gathers its 256 embedding rows (indirect DMA), PE-transposes them,
  2. precomputes xproj = W_x @ x + b for every step in one matmul, then runs
     both recurrences with one small K=16 matmul + tanh per step,
  3. single logits pass per 128-row group (rc): 1024-col bf16 matmuls into
     [128,2048] psum pairs; ACT computes exp with fused row-sum accumulation
     (accum_out) into per-pair columns while DVE copies the raw logits
     psum->SBUF as bf16 into a cached pool (no second matmul pass),
  4. after a group's sums finish: -ln(sum) per row, then the output phase
     adds it in place (ACT/DVE alternating) and DMAs each bf16 pair tile
     straight to HBM.  Group 1's compute phase is emitted interleaved with
     group 0's output phase so tensor/ACT/DVE/DMA all overlap.
Output is stored bf16 (log-probs ~[-20,-2]; bf16 rounding ~2e-3 relative,
well inside tolerance); the host upcasts to f32 and concatenates the 8
batch slices.  No collectives needed.
"""

import sys

sys.path.insert(0, "/opt/trn_rl_repo")

from contextlib import ExitStack

import numpy as np

import concourse.bass as bass
import concourse.bacc as bacc
import concourse.tile as tile
from concourse import mybir
from concourse.bass_utils import run_bass_kernel_spmd
from concourse.masks import make_identity

S, B, V, HID, EMB = 64, 32, 50257, 16, 32
NCORES = 8
BL = B // NCORES          # batch columns per core
R = S * BL                # logit rows per core
XA = EMB + 1              # 33: [x; 1] contraction for the xproj precompute
KA = 2 * HID + 1          # 33: [hLR; hRL; 1] contraction for logits
MMC = 512                 # vocab columns per matmul (ISA s3d3 limit)
PAIR = 4 * MMC            # columns per psum tile / logits tile / ACT+DVE op
NPAIR = 25                # pairs per 128-row group
VPAD = NPAIR * PAIR       # 51200; pad columns get bias -1e4 so exp == 0
WGRP = 2                  # pairs per weight-load DMA

f32 = mybir.dt.float32
bf16 = mybir.dt.bfloat16
i32 = mybir.dt.int32
AF = mybir.ActivationFunctionType


def build_nc():
    nc = bacc.Bacc()

    ind = nc.declare_dram_parameter("ind", [R, 1], i32, isOutput=False)
    emb_tab = nc.declare_dram_parameter("emb_tab", [V, EMB], f32, isOutput=False)
    # [W_x.T; b] per direction for the xproj precompute
    wlrx = nc.declare_dram_parameter("wlrx", [XA, HID], f32, isOutput=False)
    wrlx = nc.declare_dram_parameter("wrlx", [XA, HID], f32, isOutput=False)
    # W_h.T per direction for the per-step recurrence matmul
    wlrh = nc.declare_dram_parameter("wlrh", [HID, HID], f32, isOutput=False)
    wrlh = nc.declare_dram_parameter("wrlh", [HID, HID], f32, isOutput=False)
    h0c = nc.declare_dram_parameter("h0c", [HID, BL], f32, isOutput=False)
    waug_bf = nc.declare_dram_parameter("waug_bf", [KA, VPAD], bf16, isOutput=False)
    out = nc.declare_dram_parameter("out", [R, V], bf16, isOutput=True)

    with ExitStack() as ctx:
        tc = ctx.enter_context(tile.TileContext(nc))
        consts = ctx.enter_context(tc.tile_pool(name="consts", bufs=1))
        wpool = ctx.enter_context(tc.tile_pool(name="wpool", bufs=2))
        spool = ctx.enter_context(tc.tile_pool(name="spool", bufs=2))
        lpool = ctx.enter_context(tc.tile_pool(name="lpool", bufs=32))
        pbig = ctx.enter_context(tc.tile_pool(name="pbig", bufs=2, space="PSUM"))

        # ---- gather embeddings for this core's 256 (step, batch) rows ----
        idx0 = consts.tile([128, 1], i32, tag="idx0")
        idx1 = consts.tile([128, 1], i32, tag="idx1")
        nc.sync.dma_start(out=idx0[:, :], in_=ind[0:128, :])
        nc.sync.dma_start(out=idx1[:, :], in_=ind[128:256, :])
        er0 = consts.tile([128, EMB], f32, tag="er0")
        er1 = consts.tile([128, EMB], f32, tag="er1")
        nc.gpsimd.indirect_dma_start(
            out=er0[:, :], out_offset=None, in_=emb_tab[:, :],
            in_offset=bass.IndirectOffsetOnAxis(ap=idx0[:, :1], axis=0))
        nc.gpsimd.indirect_dma_start(
            out=er1[:, :], out_offset=None, in_=emb_tab[:, :],
            in_offset=bass.IndirectOffsetOnAxis(ap=idx1[:, :1], axis=0))

        # ---- transpose to x-major layout: xa = [X (32 rows); ones] ----
        ident = consts.tile([128, 128], f32, tag="ident")
        make_identity(nc, ident)
        xa = consts.tile([XA, R], f32, tag="xa")
        nc.vector.memset(xa[EMB:XA, :], 1.0)
        for half, er in ((0, er0), (1, er1)):
            pt = pbig.tile([EMB, 128], f32, tag="pb")
            nc.tensor.transpose(pt[:, :], er[:, :], ident[:, :])
            nc.vector.tensor_copy(out=xa[0:EMB, half * 128:(half + 1) * 128],
                                  in_=pt[:, :])

        # ---- RNN parameters ----
        wlrx_s = consts.tile([XA, HID], f32, tag="wlrx")
        wrlx_s = consts.tile([XA, HID], f32, tag="wrlx")
        wlrh_s = consts.tile([HID, HID], f32, tag="wlrh")
        wrlh_s = consts.tile([HID, HID], f32, tag="wrlh")
        for dst, src in ((wlrx_s, wlrx), (wrlx_s, wrlx),
                         (wlrh_s, wlrh), (wrlh_s, wrlh)):
            nc.sync.dma_start(out=dst[:, :], in_=src[:, :])

        # ---- xproj precompute: xp = W_x @ x + b for all steps ----
        xp_lr = consts.tile([HID, R], f32, tag="xp_lr")
        xp_rl = consts.tile([HID, R], f32, tag="xp_rl")
        for xp, w in ((xp_lr, wlrx_s), (xp_rl, wrlx_s)):
            pp = pbig.tile([HID, R], f32, tag="pb")
            nc.tensor.matmul(pp[:, :], lhsT=w[:, :], rhs=xa[:, :],
                             start=True, stop=True)
            nc.vector.tensor_copy(out=xp[:, :], in_=pp[:, :])

        # ---- the two recurrences ----
        # h_lr block i (cols 4i:4i+4) = hLR_pre[i]; block 0 = h0.
        # h_rl block b+1 = hRL_pre[b]; block 64 = h0 (pre-state of word 63).
        h_lr = consts.tile([HID, BL * (S + 1)], f32, tag="h_lr")
        h_rl = consts.tile([HID, BL * (S + 1)], f32, tag="h_rl")
        nc.sync.dma_start(out=h_lr[:, 0:BL], in_=h0c[:, :])
        nc.sync.dma_start(out=h_rl[:, S * BL:(S + 1) * BL], in_=h0c[:, :])
        # Interleave the two chains; the pbig ring (bufs=2) hands each chain
        # its own alternating slot, so false deps coincide with true deps.
        for i in range(S):
            w = S - 1 - i
            ps = pbig.tile([HID, BL], f32, tag="pb", name=f"pl{i}")
            nc.vector.tensor_copy(out=ps[:, :], in_=xp_lr[:, i * BL:(i + 1) * BL])
            nc.tensor.matmul(ps[:, :], lhsT=wlrh_s[:, :],
                             rhs=h_lr[:, i * BL:(i + 1) * BL],
                             start=False, stop=True, skip_group_check=True)
            nc.scalar.activation(out=h_lr[:, (i + 1) * BL:(i + 2) * BL],
                                 in_=ps[:, :], func=AF.Tanh)
            ps2 = pbig.tile([HID, BL], f32, tag="pb", name=f"pr{i}")
            nc.vector.tensor_copy(out=ps2[:, :], in_=xp_rl[:, w * BL:(w + 1) * BL])
            nc.tensor.matmul(ps2[:, :], lhsT=wrlh_s[:, :],
                             rhs=h_rl[:, (w + 1) * BL:(w + 2) * BL],
                             start=False, stop=True, skip_group_check=True)
            nc.scalar.activation(out=h_rl[:, w * BL:(w + 1) * BL],
                                 in_=ps2[:, :], func=AF.Tanh)

        # ---- h_aug = [hLR; hRL; 1] as [33, 256] (matmul lhsT layout) ----
        # Rows 16:32 aren't a legal compute-engine write target (partition
        # start must be 0/32/64/96) but DMA can write there.
        haug = consts.tile([KA, R], f32, tag="haug")
        ones = consts.tile([1, R], f32, tag="ones")
        nc.vector.memset(ones[:, :], 1.0)
        nc.vector.tensor_copy(out=haug[0:HID, :], in_=h_lr[:, 0:R])
        nc.sync.dma_start(out=haug[HID:2 * HID, :], in_=h_rl[:, BL:R + BL])
        nc.vector.tensor_copy(out=haug[2 * HID:KA, :], in_=ones[:, :])
        haug_bf = consts.tile([KA, R], bf16, tag="haug_bf")
        nc.vector.tensor_copy(out=haug_bf[:, :], in_=haug[:, :])

        # ---- logits: one matmul pass per row group; exp+rowsum fused on
        # ACT (accum_out); raw logits cached in SBUF bf16; output phase of
        # group rc-1 interleaved with compute phase of group rc ----
        sums = [consts.tile([128, NPAIR], f32, tag=f"sums{rc}", name=f"sums{rc}")
                for rc in range(2)]
        negl = [consts.tile([128, 1], f32, tag=f"ng{rc}", name=f"ng{rc}")
                for rc in range(2)]
        ltiles = {}   # (rc, pair) -> logits tile awaiting the output phase

        def emit_pass1(rc, k):
            """Matmul pair k of group rc; exp+accum on ACT; bf16 copy on DVE."""
            if k % WGRP == 0:
                wtb = wpool.tile([KA, WGRP * PAIR], bf16, tag="wtb",
                                 name=f"w{rc}_{k}")
                c0 = k * PAIR
                gw = min(WGRP * PAIR, VPAD - c0)
                nc.sync.dma_start(out=wtb[:, :gw], in_=waug_bf[:, c0:c0 + gw])
                emit_pass1.wtb = wtb
            wtb = emit_pass1.wtb
            off = (k % WGRP) * PAIR
            pt = pbig.tile([128, PAIR], f32, tag="pb", name=f"p{rc}_{k}")
            for h in range(PAIR // MMC):
                nc.tensor.matmul(pt[:, h * MMC:(h + 1) * MMC],
                                 lhsT=haug_bf[:, rc * 128:(rc + 1) * 128],
                                 rhs=wtb[:, off + h * MMC:off + (h + 1) * MMC],
                                 start=True, stop=True)
            scr = spool.tile([128, PAIR], bf16, tag="scr")
            nc.scalar.activation(out=scr[:, :], in_=pt[:, :], func=AF.Exp,
                                 accum_out=sums[rc][:, k:k + 1])
            lt = lpool.tile([128, PAIR], bf16, tag="lt", name=f"l{rc}_{k}")
            nc.vector.tensor_copy(out=lt[:, :], in_=pt[:, :])
            ltiles[(rc, k)] = lt

        def emit_lnz(rc):
            tot = consts.tile([128, 1], f32, tag=f"tot{rc}", name=f"tot{rc}")
            nc.vector.reduce_sum(out=tot[:, :], in_=sums[rc][:, 0:NPAIR],
                                 axis=mybir.AxisListType.X)
            ln = consts.tile([128, 1], f32, tag=f"ln{rc}", name=f"ln{rc}")
            nc.scalar.activation(out=ln[:, :], in_=tot[:, :], func=AF.Ln)
            nc.vector.tensor_scalar_mul(out=negl[rc][:, :], in0=ln[:, :],
                                        scalar1=-1.0)

        def emit_pass2(rc, k):
            """Add -ln(sum) in place (ACT/DVE alternating), then DMA out."""
            lt = ltiles.pop((rc, k))
            if k % 2 == 0:
                nc.vector.tensor_scalar_add(out=lt[:, :], in0=lt[:, :],
                                            scalar1=negl[rc][:, 0:1])
            else:
                nc.scalar.activation(out=lt[:, :], in_=lt[:, :],
                                     func=AF.Identity, bias=negl[rc][:, 0:1])
            c0 = k * PAIR
            cw = min(PAIR, V - c0)
            if cw > 0:
                nc.sync.dma_start(out=out[rc * 128:(rc + 1) * 128, c0:c0 + cw],
                                  in_=lt[:, :cw])

        for k in range(NPAIR):
            emit_pass1(0, k)
        emit_lnz(0)
        for k in range(NPAIR):
            emit_pass1(1, k)
            emit_pass2(0, k)
        emit_lnz(1)
        for k in range(NPAIR):
            emit_pass2(1, k)
    nc.finalize()
    return nc


_NC = None


def get_nc():
    global _NC
    if _NC is None:
        _NC = build_nc()
    return _NC


def _make_waug(Who, bho):
    # Matches the haug partition layout: [W_hLR; W_hRL; b_ho].
    # Pad columns carry bias -1e4 so exp(logit) underflows to exactly 0.
    waug = np.zeros((KA, VPAD), dtype=np.float32)
    waug[0:2 * HID, :V] = Who.T
    waug[2 * HID, :V] = bho
    waug[2 * HID, V:] = -1e4
    return waug


def make_in_maps(**inputs):
    ib = np.asarray(inputs["input_batch"]).astype(np.int32)          # [S, B]
    emb = np.ascontiguousarray(np.asarray(inputs["embedding"], dtype=np.float32))
    Wlr = np.asarray(inputs["W_lr"], dtype=np.float32)               # [16, 48]
    Wrl = np.asarray(inputs["W_rl"], dtype=np.float32)
    blr = np.asarray(inputs["b_lr"], dtype=np.float32).reshape(1, HID)
    brl = np.asarray(inputs["b_rl"], dtype=np.float32).reshape(1, HID)
    Who = np.asarray(inputs["W_ho"], dtype=np.float32)               # [V, 32]
    bho = np.asarray(inputs["b_ho"], dtype=np.float32)               # [V]
    h0 = np.asarray(inputs["h0"], dtype=np.float32)                  # [1, 16]

    waug = _make_waug(Who, bho)
    shared = dict(
        emb_tab=emb,
        waug_bf=waug.astype(mybir.dt.np(bf16)),
        wlrx=np.ascontiguousarray(np.concatenate([Wlr[:, :EMB].T, blr], axis=0)),
        wrlx=np.ascontiguousarray(np.concatenate([Wrl[:, :EMB].T, brl], axis=0)),
        wlrh=np.ascontiguousarray(Wlr[:, EMB:].T),
        wrlh=np.ascontiguousarray(Wrl[:, EMB:].T),
        h0c=np.ascontiguousarray(np.broadcast_to(h0.T, (HID, BL))),
    )
    in_maps = []
    for c in range(NCORES):
        ind = np.ascontiguousarray(
            ib[:, c * BL:(c + 1) * BL].reshape(R, 1))
        in_maps.append({**shared, "ind": ind})
    return in_maps


def assemble(results):
    outs = [results[c]["out"].astype(np.float32).reshape(S, BL, V)
            for c in range(NCORES)]
    return np.concatenate(outs, axis=1)


def kernel(**inputs):
    in_maps = make_in_maps(**inputs)
    res = run_bass_kernel_spmd(get_nc(), in_maps, list(range(NCORES)))
    return assemble(res.results)


if __name__ == "__main__":
    rng = np.random.default_rng(0)
    stdv = 1.0 / np.sqrt(HID)
    u = lambda *shp: rng.uniform(-stdv, stdv, shp).astype(np.float32)
    demo = dict(
        input_batch=rng.integers(0, V, (S, B)).astype(np.int32),
        embedding=u(V, EMB), W_lr=u(HID, EMB + HID), b_lr=u(HID),
        W_rl=u(HID, EMB + HID), b_rl=u(HID), W_ho=u(V, 2 * HID), b_ho=u(V),
        h0=u(1, HID),
    )
    out_arr = kernel(**demo)
    print(out_arr.shape, out_arr.dtype, float(out_arr[0, 0, :3].sum()))
